# revision 9
# baseline (speedup 1.0000x reference)
"""CIN (xDeepFM) 3-layer kernel for Trainium2, 8-core data parallel.

Math (per layer l, with IN = input viewed [F=64, n] and X = previous
activation [H, n], n = (b, d) flattened):
    pre[o, n] = sum_{h, f} Wl[o, h, f] * X[h, n] * IN[f, n]
    Xnext = relu(pre + bl);  out_l[o, b] = sum_d Xnext[o, (b, d)]

Device strategy per core (64 batches, n = 0..2047):
  - z k-tiles (one f x 128 h) built by VectorE bf16 tensor_mul:
    z = X (natural [h=partitions, n] layout) * table_f, where
    table_f[p, n] = IN[f, n] is a partition-broadcast DMA'd tile.
  - Layer 0 (h=64): f-pairs stacked vertically -> ordinary K=128 matmul
    (the pair sums inside the contraction).
  - Matmuls in bf16 accumulate over f into one PSUM bank [o=128, 512].
  - ScalarE applies bias+ReLU straight from PSUM (per-partition bias),
    producing the next layer's X in its natural layout. No transposes.
  - VectorE grouped reduce sums over d (innermost 32) for the output.
"""

import numpy as np
import ml_dtypes

import concourse.bass as bass
import concourse.bacc as bacc
import concourse.tile as tile
import concourse.mybir as mybir
from concourse.bass_utils import run_bass_kernel_spmd

BF16 = ml_dtypes.bfloat16

B, F, D = 512, 64, 32
NCORES = 8
BL = B // NCORES          # 64 batches per core
N = BL * D                # 2048 columns per core
CH = 512                  # chunk width (columns)
NCH = N // CH             # 4 chunks
O = 128                   # out channels per layer
GRP = 8                   # table rows per slot tile
bf16 = mybir.dt.bfloat16
f32 = mybir.dt.float32

_cache = {}


def _build_program():
    from contextlib import ExitStack

    nc = bacc.Bacc("TRN2")
    inp = nc.declare_dram_parameter("inp", [2 * F, N], bf16, isOutput=False)
    w0 = nc.declare_dram_parameter("w0", [128, 32, 128], bf16, isOutput=False)
    w1 = nc.declare_dram_parameter("w1", [128, 64, 128], bf16, isOutput=False)
    w2 = nc.declare_dram_parameter("w2", [128, 64, 128], bf16, isOutput=False)
    b0 = nc.declare_dram_parameter("b0", [128, 1], f32, isOutput=False)
    b1 = nc.declare_dram_parameter("b1", [128, 1], f32, isOutput=False)
    b2 = nc.declare_dram_parameter("b2", [128, 1], f32, isOutput=False)
    tab2 = nc.declare_dram_parameter("tab2", [NCH, 4, 128, GRP, CH], bf16,
                                     isOutput=False)
    out = nc.declare_dram_parameter("out", [3, 128, BL], f32, isOutput=True)

    with tile.TileContext(nc) as tc, ExitStack() as ctx:
        wpool = ctx.enter_context(tc.tile_pool(name="w", bufs=1))
        xpool = ctx.enter_context(tc.tile_pool(name="x0", bufs=1))
        xc_pool = ctx.enter_context(tc.tile_pool(name="xc", bufs=4))
        tabs = ctx.enter_context(tc.tile_pool(name="tabs", bufs=16))
        zpool = ctx.enter_context(tc.tile_pool(name="z", bufs=6))
        opool = ctx.enter_context(tc.tile_pool(name="oacc", bufs=1))
        pspool = ctx.enter_context(tc.tile_pool(name="ps", bufs=3, space="PSUM"))

        # resident weights [p=k-row, f-slot, o]
        w0_t = wpool.tile([128, 32, 128], bf16)
        nc.sync.dma_start(w0_t[:], w0[:])
        w1_t = wpool.tile([128, 64, 128], bf16)
        nc.sync.dma_start(w1_t[:], w1[:])
        w2_t = wpool.tile([128, 64, 128], bf16)
        nc.sync.dma_start(w2_t[:], w2[:])
        b0_t = wpool.tile([128, 1], f32)
        nc.sync.dma_start(b0_t[:], b0[:])
        b1_t = wpool.tile([128, 1], f32)
        nc.sync.dma_start(b1_t[:], b1[:])
        b2_t = wpool.tile([128, 1], f32)
        nc.sync.dma_start(b2_t[:], b2[:])
        bias_ts = [b0_t, b1_t, b2_t]

        # X0 stacked twice: [IN; IN] so partition p holds IN[p mod 64]
        x0_t = xpool.tile([128, N], bf16)
        nc.sync.dma_start(x0_t[:], inp[:])

        oacc = [opool.tile([128, BL], f32, name=f"oacc{i}", tag=f"oacc{i}")
                for i in range(3)]

        for c in range(NCH):
            ns = c * CH

            # ---- layer-0 tables: slot[p, j, n] = IN[2*(8g+j) + p//64, n]
            t2 = []
            for g in range(4):
                s = tabs.tile([128, GRP, CH], bf16, tag="tab")
                nc.sync.dma_start(s[:], tab2[c, g])
                t2.append(s)
            # ---- layer-1/2 tables: slot[p, j, n] = IN[8g+j, n]
            t1 = []
            for g in range(8):
                s = tabs.tile([128, GRP, CH], bf16, tag="tab")
                nc.sync.dma_start(
                    s[:, :, :],
                    inp[8 * g:8 * g + 8, ns:ns + CH].partition_broadcast(128))
                t1.append(s)

            # ---- layer 0: 32 f-pair k-tiles
            ps0 = pspool.tile([128, CH], f32)
            for fp in range(32):
                z = zpool.tile([128, CH], bf16, tag="z")
                nc.vector.tensor_mul(z[:], x0_t[:, ns:ns + CH],
                                     t2[fp // GRP][:, fp % GRP, :])
                nc.tensor.matmul(ps0[:], w0_t[:, fp, :], z[:],
                                 start=(fp == 0), stop=(fp == 31))
            x1c = xc_pool.tile([128, CH], bf16, tag="xc")
            nc.scalar.activation(x1c[:], ps0[:],
                                 mybir.ActivationFunctionType.Relu,
                                 bias=bias_ts[0], scale=1.0)
            nc.vector.tensor_reduce(
                oacc[0][:, c * (CH // D):(c + 1) * (CH // D)],
                x1c.rearrange("p (g d) -> p g d", d=D),
                axis=mybir.AxisListType.X, op=mybir.AluOpType.add)

            # ---- layers 1 and 2
            xin = x1c
            for li, (w_t, ps_i) in enumerate(((w1_t, 1), (w2_t, 2))):
                ps = pspool.tile([128, CH], f32)
                for f in range(64):
                    z = zpool.tile([128, CH], bf16, tag="z")
                    nc.vector.tensor_mul(z[:], xin[:],
                                         t1[f // GRP][:, f % GRP, :])
                    nc.tensor.matmul(ps[:], w_t[:, f, :], z[:],
                                     start=(f == 0), stop=(f == 63))
                xo = xc_pool.tile([128, CH], bf16, tag="xc")
                nc.scalar.activation(xo[:], ps[:],
                                     mybir.ActivationFunctionType.Relu,
                                     bias=bias_ts[ps_i], scale=1.0)
                nc.vector.tensor_reduce(
                    oacc[ps_i][:, c * (CH // D):(c + 1) * (CH // D)],
                    xo.rearrange("p (g d) -> p g d", d=D),
                    axis=mybir.AxisListType.X, op=mybir.AluOpType.add)
                xin = xo

        for li in range(3):
            nc.sync.dma_start(out[li], oacc[li][:])

    nc.finalize()
    return nc


def _pack_weights(W0, b0, W1, b1, W2, b2):
    # W*.reshape(o, h, f) -> stationary [p=k-row, f-slot, o]
    W0r = W0.reshape(128, 64, 64)
    w0p = np.ascontiguousarray(
        W0r.transpose(2, 1, 0)              # [f, h, o]
        .reshape(32, 2, 64, 128)            # [fp, df, h, o]
        .reshape(32, 128, 128)              # [fp, p=df*64+h, o]
        .transpose(1, 0, 2)).astype(BF16)   # [p, fp, o]
    w1p = np.ascontiguousarray(
        W1.reshape(128, 128, 64).transpose(2, 1, 0)   # [f, h, o]
        .transpose(1, 0, 2)).astype(BF16)             # [h, f, o]
    w2p = np.ascontiguousarray(
        W2.reshape(128, 128, 64).transpose(2, 1, 0)
        .transpose(1, 0, 2)).astype(BF16)
    return {
        "w0": w0p, "w1": w1p, "w2": w2p,
        "b0": np.asarray(b0, np.float32).reshape(128, 1),
        "b1": np.asarray(b1, np.float32).reshape(128, 1),
        "b2": np.asarray(b2, np.float32).reshape(128, 1),
    }


def make_in_maps(input, W0, b0, W1, b1, W2, b2):
    shared = _pack_weights(np.asarray(W0), np.asarray(b0), np.asarray(W1),
                           np.asarray(b1), np.asarray(W2), np.asarray(b2))
    in_maps = []
    inp_np = np.asarray(input)
    for c in range(NCORES):
        shard = inp_np[c * BL:(c + 1) * BL]          # [BL, F, D]
        IN = np.ascontiguousarray(
            shard.transpose(1, 0, 2).reshape(F, N)).astype(BF16)
        INs = np.ascontiguousarray(np.concatenate([IN, IN], axis=0))
        # layer-0 table: tab2[c, g, p, j, n] = IN[2*(8g+j) + p//64, c*CH+n]
        INf = IN.reshape(F, NCH, CH)
        ev = np.transpose(INf[0::2].reshape(4, GRP, NCH, CH), (2, 0, 1, 3))
        od = np.transpose(INf[1::2].reshape(4, GRP, NCH, CH), (2, 0, 1, 3))
        t2a = np.empty((NCH, 4, 128, GRP, CH), BF16)
        t2a[:, :, 0:64] = ev[:, :, None, :, :]
        t2a[:, :, 64:128] = od[:, :, None, :, :]
        in_maps.append({"inp": INs, "tab2": t2a, **shared})
    return in_maps


def gather_out(results):
    # per-core out [3, 128, BL] -> full [B, 384]
    return np.concatenate(
        [np.asarray(r["out"], np.float32).transpose(2, 0, 1).reshape(BL, 3 * O)
         for r in results], axis=0)


def kernel(input, W0, b0, W1, b1, W2, b2):
    if "nc" not in _cache:
        _cache["nc"] = _build_program()
    nc = _cache["nc"]
    in_maps = make_in_maps(input, W0, b0, W1, b1, W2, b2)
    res = run_bass_kernel_spmd(nc, in_maps, list(range(NCORES)))
    return gather_out(res.results)
